# revision 9
# baseline (speedup 1.0000x reference)
"""BiLSTM-CRF NLL loss on 8 Trainium2 NeuronCores — v2.

Sharding: core c owns sequences [4c, 4c+4); each core runs BOTH LSTM
directions locally (fwd chain + bwd chain, software-pipelined) and the full
CRF for its 4 sequences. No collectives; host sums the 8 per-core partials.

Orientation: gates live on PSUM partitions (gate-unit-major), batch on the
free dim, so each recurrence matmul moves only 4 columns. Per step and chain:
16 gate-chunk tiles x (1 gx-inject + 4 W_hh k-chunk) matmuls -> Act
sigmoid(i,f,o) + tanh(g) -> DVE cell update -> Act tanh(c) -> DVE h.
h is produced directly in transposed (unit-major) layout: no transposes.

Self-contained: hardcodes all shapes; only needs numpy + concourse (+ml_dtypes).
"""
import numpy as np
import ml_dtypes

import concourse.bass as bass
import concourse.bacc as bacc
import concourse.tile as tile
from concourse import mybir
from concourse.tile_rust import add_dep_helper
from concourse.bass_utils import run_bass_kernel_spmd

F32 = mybir.dt.float32
FP8 = mybir.dt.float8e4
BF16 = mybir.dt.bfloat16
I32 = mybir.dt.int32
AF = mybir.ActivationFunctionType
ALU = mybir.AluOpType

B, S, E, H, T, V = 32, 256, 256, 512, 45, 50000
NS = 4                 # seqs per core
N = S * NS             # 1024 emission cols, n = 4t+s
NCH = 16               # gate chunks (2048/128)
HC = 4                 # h chunks (512/128)
SW = HC * NS           # state cols per step = 16
NB_T = 4               # transform n-blocks (of 64 steps = 256 cols each)
TBLK = S // NB_T       # 64 steps per transform block
LN45 = float(np.log(45.0))

_cached = {}


def _build(stop_after=None):
    lv = {"xf": 1, "rec": 2, "em": 3, "crf": 4, None: 5}[stop_after]
    nc = bacc.Bacc("TRN2", target_bir_lowering=False, debug=False, num_devices=8)

    d = {}
    d["emb"] = nc.dram_tensor("emb", [V, E], BF16, kind="ExternalInput")
    d["xidx"] = nc.dram_tensor("xidx", [128, 8], I32, kind="ExternalInput")
    d["wihf"] = nc.dram_tensor("wihf", [128, 32 * 128], FP8, kind="ExternalInput")
    d["wihb"] = nc.dram_tensor("wihb", [128, 32 * 128], FP8, kind="ExternalInput")
    d["whhf"] = nc.dram_tensor("whhf", [128, 64 * 128], FP8, kind="ExternalInput")
    d["whhb"] = nc.dram_tensor("whhb", [128, 64 * 128], FP8, kind="ExternalInput")
    d["biasf"] = nc.dram_tensor("biasf", [1, 2048], F32, kind="ExternalInput")
    d["biasb"] = nc.dram_tensor("biasb", [1, 2048], F32, kind="ExternalInput")
    d["linT"] = nc.dram_tensor("linT", [128, 8 * T], BF16, kind="ExternalInput")
    d["linb"] = nc.dram_tensor("linb", [T, 1], F32, kind="ExternalInput")
    d["id128"] = nc.dram_tensor("id128", [128, 128], F32, kind="ExternalInput")
    d["idbf"] = nc.dram_tensor("idbf", [128, 128], BF16, kind="ExternalInput")
    d["trans"] = nc.dram_tensor("trans", [T, T], F32, kind="ExternalInput")
    d["stend"] = nc.dram_tensor("stend", [T, 2], F32, kind="ExternalInput")
    d["oh"] = nc.dram_tensor("oh", [T, N], F32, kind="ExternalInput")
    d["oh2"] = nc.dram_tensor("oh2", [T, N], F32, kind="ExternalInput")
    d_loss = nc.dram_tensor("loss", [1, NS], F32, kind="ExternalOutput")

    with tile.TileContext(nc) as tc:
        with tc.tile_pool(name="persist", bufs=1) as pp, \
             tc.tile_pool(name="gxp", bufs=1) as gxp:
            # persistent weights / tables
            wih = {0: pp.tile([128, 32 * 128], FP8, tag="wihf", name="wihf"),
                   1: pp.tile([128, 32 * 128], FP8, tag="wihb", name="wihb")}
            whh = {0: pp.tile([128, 64 * 128], FP8, tag="whhf", name="whhf"),
                   1: pp.tile([128, 64 * 128], FP8, tag="whhb", name="whhb")}
            bias = {0: pp.tile([1, 2048], F32, tag="biasf", name="biasf"),
                    1: pp.tile([1, 2048], F32, tag="biasb", name="biasb")}
            ones1 = pp.tile([1, NS], F32, tag="ones1")
            id128 = pp.tile([128, 128], F32, tag="id128")
            idbf = pp.tile([128, 128], BF16, tag="idbf")
            xidx = pp.tile([128, 8], I32, tag="xidx")
            linT = pp.tile([128, 8 * T], BF16, tag="linT")
            nc.sync.dma_start(out=xidx[:], in_=d["xidx"][:])
            nc.sync.dma_start(out=id128[:], in_=d["id128"][:])
            nc.sync.dma_start(out=idbf[:], in_=d["idbf"][:])
            nc.vector.memset(ones1[:], 1.0)

            # XT block tiles: [nb] -> [128, 2 ec x 256 n] bf16
            xt = {nb: gxp.tile([128, 2 * TBLK * NS], FP8, tag=f"xt{nb}", name=f"xt{nb}")
                  for nb in range(NB_T)}
            # h state (unit-major): slot p in 0..255 = position, slot 256 = h0
            hsT = {0: pp.tile([128, SW * (S + 1)], FP8, tag="hsTf", name="hsTf"),
                   1: pp.tile([128, SW * (S + 1)], FP8, tag="hsTb", name="hsTb")}
            nc.vector.memset(hsT[0][:, SW * S: SW * (S + 1)], 0.0)
            nc.vector.memset(hsT[1][:, SW * S: SW * (S + 1)], 0.0)

            # ---------- phase 0: gather + transpose -> XT ----------
            with tc.tile_pool(name="gat", bufs=3) as gp, \
                 tc.tile_pool(name="ps_tp", bufs=4, space="PSUM") as ps_tp:
                for b in range(8):
                    X = gp.tile([128, E], BF16, tag="X")
                    nc.gpsimd.indirect_dma_start(
                        out=X[:],
                        out_offset=None,
                        in_=d["emb"][:],
                        in_offset=bass.IndirectOffsetOnAxis(ap=xidx[:, b:b + 1], axis=0),
                    )
                    nb, off = b // 2, (b % 2) * 128
                    for ec in range(2):
                        tp = ps_tp.tile([128, 128], BF16, tag="tp")
                        nc.tensor.transpose(tp[:], X[:, 128 * ec: 128 * ec + 128], idbf[:])
                        nc.vector.tensor_copy(
                            xt[nb][:, TBLK * NS * ec + off: TBLK * NS * ec + off + 128],
                            tp[:])

            # weight DMAs after the gathers so they share the DMA engines
            nc.sync.dma_start(out=wih[0][:], in_=d["wihf"][:])
            nc.sync.dma_start(out=wih[1][:], in_=d["wihb"][:])
            nc.sync.dma_start(out=whh[0][:], in_=d["whhf"][:])
            nc.sync.dma_start(out=whh[1][:], in_=d["whhb"][:])
            nc.sync.dma_start(out=bias[0][:], in_=d["biasf"][:])
            nc.sync.dma_start(out=bias[1][:], in_=d["biasb"][:])
            nc.sync.dma_start(out=linT[:], in_=d["linT"][:])

            # ---------- recurrence (x-transform fused into gate matmuls) ----------
            if lv == 1:
                probe = pp.tile([1, NS], F32, tag="probe")
                nc.vector.tensor_copy(probe[:], xt[0][0:1, 0:NS])
                nc.sync.dma_start(out=d_loss[:], in_=probe[:])
            if True:
                if lv >= 2:
                    with tc.tile_pool(name="rec0", bufs=4) as rp0, \
                         tc.tile_pool(name="rec1", bufs=4) as rp1, \
                         tc.tile_pool(name="psg0", bufs=2, space="PSUM") as pg0, \
                         tc.tile_pool(name="psg1", bufs=2, space="PSUM") as pg1, \
                         tc.tile_pool(name="psi0", bufs=2, space="PSUM") as pi0, \
                         tc.tile_pool(name="psi1", bufs=2, space="PSUM") as pi1:
                        rp = [rp0, rp1]
                        pg = [pg0, pg1]
                        pi = [pi0, pi1]
                        cprev = [None, None]
                        for dd in (0, 1):
                            cinit = rp[dd].tile([128, SW], BF16, tag="c")
                            nc.vector.memset(cinit[:], 0.0)
                            cprev[dd] = cinit

                        def rstep(dd, u):
                            # slot map:
                            # fwd: h_f(u) -> slot u; reads h_f(u-1) at slot u-1
                            #      (u=0 reads slot S = zeros)
                            # bwd: h_b(p=S-1-u) -> slot p; reads slot p+1
                            #      (u=0 reads slot S = zeros)
                            if dd == 0:
                                slot_w = u
                                slot_r = S if u == 0 else u - 1
                                col = u            # gx col index (timestep)
                            else:
                                p = S - 1 - u
                                slot_w = p
                                slot_r = S if u == 0 else p + 1
                                col = p
                            nb, j = col // TBLK, col % TBLK
                            xtb = xt[nb]
                            hprev = hsT[dd][:, SW * slot_r: SW * slot_r + SW]
                            Gg = pg[dd].tile([128, SW], F32, tag="Gg")
                            Gifo = pi[dd].tile([128, 3 * SW], F32, tag="Gi")

                            # gate chunks: i=0:4, f=4:8, o=8:12 (Gifo tile,
                            # cols 4*m), g=12:16 (Gg tile). One PSUM
                            # accumulation group per physical tile (= one 2KB
                            # zero region): start=True only on the tile's very
                            # first mm, stop=True on its very last; all other
                            # mms accumulate (first touch of each address
                            # replaces, since start marks the whole region
                            # pending-zero). Phase A (bias + W_ih x, no h dep)
                            # is emitted before phase B (W_hh h) so it can run
                            # in the previous step's tail; add_dep_helper pins
                            # start-first / stop-last against scheduler
                            # reordering. u=0: h_prev = 0, phase B skipped.
                            tiles = [(Gg, list(range(12, 16))),
                                     (Gifo, list(range(12)))]
                            DR = mybir.MatmulPerfMode.DoubleRow
                            xtv = xtb.rearrange("p (e c) -> p e c", e=2)
                            tile_mms = [[] for _ in tiles]
                            for gi, (dst, mlist) in enumerate(tiles):
                                for pos, m in enumerate(mlist):
                                    sl = dst[:, NS * pos: NS * pos + NS]
                                    mm = nc.tensor.matmul(
                                        sl, bias[dd][:, 128 * m: 128 * (m + 1)],
                                        ones1[:], start=(pos == 0), stop=False)
                                    tile_mms[gi].append(mm)
                                    # both E-chunks in one DoubleRow mm
                                    wpair = wih[dd][:, 2 * m * 128:
                                                    (2 * m + 2) * 128]
                                    mm = nc.tensor.matmul(
                                        sl,
                                        wpair.rearrange("p (c f) -> p c f", c=2),
                                        xtv[:, :, NS * j: NS * j + NS],
                                        start=False,
                                        stop=(u == 0 and pos == len(mlist) - 1),
                                        perf_mode=DR)
                                    tile_mms[gi].append(mm)
                            if u > 0:
                                for gi, (dst, mlist) in enumerate(tiles):
                                    for pos, m in enumerate(mlist):
                                        sl = dst[:, NS * pos: NS * pos + NS]
                                        for kp in range(HC // 2):
                                            # k-chunks 2kp, 2kp+1 in one mm
                                            hpair = whh[dd][
                                                :, (4 * m + 2 * kp) * 128:
                                                (4 * m + 2 * kp + 2) * 128]
                                            mm = nc.tensor.matmul(
                                                sl,
                                                hpair.rearrange(
                                                    "p (c f) -> p c f", c=2),
                                                hprev[:, 8 * kp: 8 * kp + 8]
                                                .rearrange("p (c s) -> p c s",
                                                           c=2),
                                                start=False,
                                                stop=(pos == len(mlist) - 1
                                                      and kp == HC // 2 - 1),
                                                perf_mode=DR)
                                            tile_mms[gi].append(mm)
                            for mms in tile_mms:
                                first, last = mms[0], mms[-1]
                                for mm in mms[1:]:
                                    add_dep_helper(mm.ins, first.ins, sync=False,
                                                   reason="group start first")
                                for mm in mms[:-1]:
                                    add_dep_helper(last.ins, mm.ins, sync=False,
                                                   reason="group stop last")
                            SGg = rp[dd].tile([128, SW], BF16, tag="SGg")
                            SGifo = rp[dd].tile([128, 3 * SW], BF16, tag="SGi")
                            nc.scalar.activation(SGg[:], Gg[:], AF.Tanh)
                            nc.scalar.activation(SGifo[:], Gifo[:], AF.Sigmoid)
                            t1 = rp[dd].tile([128, SW], BF16, tag="t1")
                            t2 = rp[dd].tile([128, SW], BF16, tag="t2")
                            th = rp[dd].tile([128, SW], BF16, tag="th")
                            cnext = rp[dd].tile([128, SW], BF16, tag="c")
                            nc.vector.tensor_mul(t1[:], SGifo[:, SW:2 * SW], cprev[dd][:])
                            nc.vector.tensor_mul(t2[:], SGifo[:, 0:SW], SGg[:])
                            nc.vector.tensor_add(cnext[:], t1[:], t2[:])
                            nc.scalar.activation(th[:], cnext[:], AF.Tanh)
                            nc.vector.tensor_mul(
                                hsT[dd][:, SW * slot_w: SW * slot_w + SW],
                                SGifo[:, 2 * SW:3 * SW], th[:])
                            cprev[dd] = cnext

                        for u in range(S):
                            rstep(0, u)
                            rstep(1, u)

                    if lv == 2:
                        probe = pp.tile([1, NS], F32, tag="probe")
                        nc.vector.tensor_copy(probe[:], hsT[0][0:1, 0:NS])
                        nc.sync.dma_start(out=d_loss[:], in_=probe[:])

            # ---------- emissions ----------
            em_lin = pp.tile([T, N], F32, tag="em_lin")
            exp_em = pp.tile([T, N], F32, tag="exp_em")
            if lv >= 3:
                with tc.tile_pool(name="emc", bufs=1) as ec_, \
                     tc.tile_pool(name="ps_em", bufs=2, space="PSUM") as ps_em:
                    linb = ec_.tile([T, 1], F32, tag="linb")
                    nc.sync.dma_start(out=linb[:], in_=d["linb"][:])
                    hv = {dd: hsT[dd].rearrange("p (t k s) -> p t k s", k=HC, s=NS)
                          for dd in (0, 1)}
                    for nb in range(2):
                        pe = ps_em.tile([T, 512], F32, tag="pe")
                        toff = nb * 128
                        for kc in range(8):
                            dd, k = kc // 4, kc % 4
                            # rhs: hsT[dd] cols {16*(toff+t) + 4k + s}, 512 free
                            rhs = hv[dd][:, toff:toff + 128, k:k + 1, :]
                            nc.tensor.matmul(
                                pe[:], linT[:, T * kc: T * (kc + 1)], rhs,
                                start=(kc == 0), stop=(kc == 7))
                        nc.vector.tensor_scalar_add(
                            em_lin[:, 512 * nb: 512 * (nb + 1)], pe[:], linb[:])
                        nc.scalar.activation(exp_em[:, 512 * nb: 512 * (nb + 1)],
                                             pe[:], AF.Exp, bias=linb[:])
                if lv == 3:
                    probe = pp.tile([1, NS], F32, tag="probe")
                    nc.vector.tensor_copy(probe[:], em_lin[0:1, 0:NS])
                    nc.sync.dma_start(out=d_loss[:], in_=probe[:])

            # ---------- CRF ----------
            if lv >= 4:
                with tc.tile_pool(name="crf", bufs=1) as cp, \
                     tc.tile_pool(name="qs", bufs=3) as qp, \
                     tc.tile_pool(name="ps_q", bufs=2, space="PSUM") as ps_q:
                    trans_sb = cp.tile([T, T], F32, tag="trans")
                    stend = cp.tile([T, 2], F32, tag="stend")
                    Ep = cp.tile([T, T], F32, tag="Ep")
                    estart = cp.tile([T, 1], F32, tag="estart")
                    eend = cp.tile([T, 1], F32, tag="eend")
                    nln45 = cp.tile([T, 1], F32, tag="nln45")
                    ones45 = cp.tile([T, 1], F32, tag="ones45")
                    oh = cp.tile([T, N], F32, tag="oh")
                    oh2 = cp.tile([T, N], F32, tag="oh2")
                    nc.sync.dma_start(out=trans_sb[:], in_=d["trans"][:])
                    nc.sync.dma_start(out=stend[:], in_=d["stend"][:])
                    nc.sync.dma_start(out=oh[:], in_=d["oh"][:])
                    nc.sync.dma_start(out=oh2[:], in_=d["oh2"][:])
                    nc.vector.memset(nln45[:], -LN45)
                    nc.vector.memset(ones45[:], 1.0)
                    nc.scalar.activation(Ep[:], trans_sb[:], AF.Exp, bias=nln45[:])
                    nc.scalar.activation(estart[:], stend[:, 0:1], AF.Exp)
                    nc.scalar.activation(eend[:], stend[:, 1:2], AF.Exp)

                    # partition function via two-sided vector chains that
                    # meet at K=127:  Z = sum_i alpha_K(i) * beta_K(i).
                    # alpha ascends from t=0, beta descends from t=255
                    # (beta_255 = exp(end_t)); both run concurrently, halving
                    # the serial chain length. bf16 operands (4x cheaper PE
                    # streaming; ~1e-4 relative effect on logZ).
                    K = 127
                    Epb = cp.tile([T, T], BF16, tag="Epb")
                    nc.scalar.activation(Epb[:], trans_sb[:], AF.Exp, bias=nln45[:])
                    EpbT = cp.tile([T, T], BF16, tag="EpbT")
                    with tc.tile_pool(name="ps_t", bufs=1, space="PSUM") as ps_t:
                        tpt = ps_t.tile([T, T], BF16, tag="tpt")
                        nc.tensor.transpose(tpt[:], Epb[:], idbf[0:T, 0:T])
                        nc.vector.tensor_copy(EpbT[:], tpt[:])

                    q = qp.tile([T, NS], BF16, tag="q")
                    nc.vector.tensor_scalar_mul(q[:], exp_em[:, 0:NS], estart[:])
                    bq0 = qp.tile([T, NS], BF16, tag="bq")
                    nc.vector.tensor_scalar_mul(
                        bq0[:], eend[:].to_broadcast([T, NS]), ones45[:])
                    bq = bq0                     # beta lives in PSUM after j=1
                    with tc.tile_pool(name="ps_b", bufs=2, space="PSUM") as ps_b:
                        for j in range(1, K + 1):
                            # alpha: t = j
                            sA = ps_q.tile([T, NS], F32, tag="sA")
                            nc.tensor.matmul(sA[:], Epb[:], q[:],
                                             start=True, stop=True)
                            qn = qp.tile([T, NS], BF16, tag="q")
                            nc.vector.tensor_mul(
                                qn[:], sA[:], exp_em[:, NS * j: NS * (j + 1)])
                            q = qn
                            # beta: t = 255 - j; wv = e_{t+1} (.) beta_{t+1};
                            # beta_t = Ep @ wv  (lhsT = Ep^T); beta stays in
                            # PSUM, read directly by the next step's DVE mul
                            t_ = S - 1 - j
                            wv = qp.tile([T, NS], BF16, tag="wv")
                            nc.vector.tensor_mul(
                                wv[:], bq[:],
                                exp_em[:, NS * (t_ + 1): NS * (t_ + 2)])
                            sB = ps_b.tile([T, NS], F32, tag="sB")
                            nc.tensor.matmul(sB[:], EpbT[:], wv[:],
                                             start=True, stop=True)
                            bq = sB
                        # S-1=255 links is odd: one extra beta step so beta
                        # reaches position K (= alpha's position) exactly
                        wv = qp.tile([T, NS], BF16, tag="wv")
                        nc.vector.tensor_mul(
                            wv[:], bq[:], exp_em[:, NS * (K + 1): NS * (K + 2)])
                        sB = ps_b.tile([T, NS], F32, tag="sB")
                        nc.tensor.matmul(sB[:], EpbT[:], wv[:],
                                         start=True, stop=True)
                        bqf = cp.tile([T, NS], F32, tag="bqf")
                        nc.vector.tensor_copy(bqf[:], sB[:])
                        bq = bqf
                    if lv == 4:
                        probe = pp.tile([1, NS], F32, tag="probe")
                        nc.vector.tensor_copy(probe[:], q[0:1, :])
                        nc.sync.dma_start(out=d_loss[:], in_=probe[:])

                    if lv >= 5:
                        w = cp.tile([T, NS], F32, tag="w")
                        logZ = cp.tile([1, NS], F32, tag="logZ")
                        em_h = cp.tile([1, 2 * NS], F32, tag="em_h")
                        tr_h = cp.tile([1, 2 * NS], F32, tag="tr_h")
                        em_sc = cp.tile([1, NS], F32, tag="em_sc")
                        tr_sc = cp.tile([1, NS], F32, tag="tr_sc")
                        sten_s = cp.tile([1, NS], F32, tag="sten_s")
                        nc.vector.tensor_mul(w[:], q[:], bq[:])
                        with tc.tile_pool(name="ps_f", bufs=1, space="PSUM") as ps_f:
                            sumw = ps_f.tile([1, NS], F32, tag="f1")
                            nc.tensor.matmul(sumw[:], ones45[:], w[:],
                                             start=True, stop=True)
                            nc.scalar.activation(logZ[:], sumw[:], AF.Ln)

                            S1 = cp.tile([T, N], F32, tag="S1")
                            nc.vector.tensor_mul(S1[:], em_lin[:], oh[:])
                            S2 = cp.tile([T, N], F32, tag="S2")
                            for ck in range(2):
                                sl = slice(512 * ck, 512 * (ck + 1))
                                s1p = ps_f.tile([1, 512], F32, tag="fbig")
                                nc.tensor.matmul(s1p[:], ones45[:], S1[:, sl],
                                                 start=True, stop=True)
                                nc.vector.tensor_reduce(
                                    em_h[:, NS * ck: NS * (ck + 1)],
                                    s1p.rearrange("p (t b) -> p b t", b=NS),
                                    axis=mybir.AxisListType.X, op=ALU.add)
                                Rp_ = ps_f.tile([T, 512], F32, tag="fR")
                                nc.tensor.matmul(Rp_[:], trans_sb[:], oh[:, sl],
                                                 start=True, stop=True)
                                nc.vector.tensor_mul(S2[:, sl], Rp_[:], oh2[:, sl])
                                s2p = ps_f.tile([1, 512], F32, tag="fbig2")
                                nc.tensor.matmul(s2p[:], ones45[:], S2[:, sl],
                                                 start=True, stop=True)
                                nc.vector.tensor_reduce(
                                    tr_h[:, NS * ck: NS * (ck + 1)],
                                    s2p.rearrange("p (t b) -> p b t", b=NS),
                                    axis=mybir.AxisListType.X, op=ALU.add)
                            nc.vector.tensor_add(em_sc[:], em_h[:, 0:NS],
                                                 em_h[:, NS:2 * NS])
                            nc.vector.tensor_add(tr_sc[:], tr_h[:, 0:NS],
                                                 tr_h[:, NS:2 * NS])

                            stp = cp.tile([T, NS], F32, tag="stp")
                            enp = cp.tile([T, NS], F32, tag="enp")
                            nc.vector.tensor_scalar_mul(stp[:], oh[:, 0:NS],
                                                        stend[:, 0:1])
                            nc.vector.tensor_scalar_mul(enp[:], oh[:, N - NS:N],
                                                        stend[:, 1:2])
                            sten = ps_f.tile([1, NS], F32, tag="f2")
                            nc.tensor.matmul(sten[:], ones45[:], stp[:],
                                             start=True, stop=False)
                            nc.tensor.matmul(sten[:], ones45[:], enp[:],
                                             start=False, stop=True)
                            nc.vector.tensor_copy(sten_s[:], sten[:])

                        sc1 = cp.tile([1, NS], F32, tag="sc1")
                        sc2 = cp.tile([1, NS], F32, tag="sc2")
                        lossa = cp.tile([1, NS], F32, tag="lossa")
                        lossb = cp.tile([1, NS], F32, tag="lossb")
                        nc.vector.tensor_add(sc1[:], em_sc[:], tr_sc[:])
                        nc.vector.tensor_add(sc2[:], sc1[:], sten_s[:])
                        nc.vector.tensor_tensor(out=lossa[:], in0=logZ[:],
                                                in1=sc2[:], op=ALU.subtract)
                        nc.scalar.activation(lossb[:], lossa[:], AF.Copy,
                                             bias=(S - 1) * LN45)
                        nc.sync.dma_start(out=d_loss[:], in_=lossb[:])

    nc.finalize()
    return nc


def _pack_wT(w, kchunks):
    # w: [M_out=2048-ish rows (gate units, reordered), K] ->
    # [128, (nm*kchunks)*128] tiles: tile (m*kchunks+ec) = w[mU, ecK].T
    M, K = w.shape
    nm = M // 128
    assert K == 128 * kchunks
    tiles = []
    for m in range(nm):
        for ec in range(kchunks):
            blk = w[m * 128:(m + 1) * 128, ec * 128:(ec + 1) * 128]
            tiles.append(np.ascontiguousarray(blk.T))
    return np.concatenate(tiles, axis=1)


def _perm_gates_ifog(w):
    # torch gate order i,f,g,o (blocks of H) -> our chunk order i,f,o,g
    i, f, g, o = np.split(w, 4, axis=0)
    return np.concatenate([i, f, o, g], axis=0)


def prepare_in_maps(**inputs):
    x = np.asarray(inputs["x"]).astype(np.int32)          # [32, 256]
    tags = np.asarray(inputs["tags"]).astype(np.int32)
    emb = np.asarray(inputs["emb"], dtype=np.float32)
    lin_w = np.asarray(inputs["lin_w"], dtype=np.float32)
    lin_b = np.asarray(inputs["lin_b"], dtype=np.float32)
    start_t = np.asarray(inputs["start_t"], dtype=np.float32)
    end_t = np.asarray(inputs["end_t"], dtype=np.float32)
    trans = np.asarray(inputs["trans"], dtype=np.float32)

    wihp = {0: _perm_gates_ifog(np.asarray(inputs["w_ih_f"], np.float32)),
            1: _perm_gates_ifog(np.asarray(inputs["w_ih_b"], np.float32))}
    whhp = {0: _perm_gates_ifog(np.asarray(inputs["w_hh_f"], np.float32)),
            1: _perm_gates_ifog(np.asarray(inputs["w_hh_b"], np.float32))}
    bp = {0: _perm_gates_ifog(np.asarray(inputs["b_f"], np.float32)),
          1: _perm_gates_ifog(np.asarray(inputs["b_b"], np.float32))}

    wih_t = {dd: _pack_wT(wihp[dd], 2).astype(ml_dtypes.float8_e4m3) for dd in (0, 1)}
    whh_t = {dd: _pack_wT(whhp[dd], 4).astype(ml_dtypes.float8_e4m3) for dd in (0, 1)}
    bias_t = {dd: bp[dd].reshape(1, 2048) for dd in (0, 1)}

    # linT [128, 8*T]: tile kc = lin_w[:, kc*128:(kc+1)*128].T (fwd 0-3, bwd 4-7)
    lin_tiles = [np.ascontiguousarray(lin_w[:, kc * 128:(kc + 1) * 128].T)
                 for kc in range(8)]
    linT = np.concatenate(lin_tiles, axis=1).astype(ml_dtypes.bfloat16)

    id128 = np.eye(128, dtype=np.float32)

    in_maps = []
    for core in range(8):
        seqs = slice(4 * core, 4 * core + 4)
        xs = x[seqs]                                      # [4, 256]
        # xidx [128, 16]: col b, row r -> x[s=(r%4), t=(128b+r)//4]
        nflat = xs.T.reshape(-1)                          # n = 4t+s
        xidx = np.ascontiguousarray(nflat.reshape(8, 128).T).astype(np.int32)

        tg = tags[seqs]                                   # [4, 256]
        oh = np.zeros((T, N), np.float32)
        oh[tg.T.reshape(-1), np.arange(N)] = 1.0
        oh2 = np.zeros((T, N), np.float32)
        oh2[:, 0:N - NS] = oh[:, NS:N]

        in_maps.append({
            "emb": emb.astype(ml_dtypes.bfloat16),
            "xidx": xidx,
            "wihf": wih_t[0], "wihb": wih_t[1],
            "whhf": whh_t[0], "whhb": whh_t[1],
            "biasf": bias_t[0], "biasb": bias_t[1],
            "linT": linT,
            "linb": lin_b.reshape(T, 1),
            "id128": id128,
            "idbf": np.eye(128, dtype=ml_dtypes.bfloat16),
            "trans": trans,
            "stend": np.stack([start_t, end_t], axis=1),
            "oh": oh,
            "oh2": oh2,
        })
    return in_maps


def get_nc():
    if "nc" not in _cached:
        _cached["nc"] = _build()
    return _cached["nc"]


def kernel(**inputs):
    in_maps = prepare_in_maps(**inputs)
    res = run_bass_kernel_spmd(get_nc(), in_maps, core_ids=list(range(8)))
    total = np.float64(0.0)
    for core in range(8):
        total += np.float64(res.results[core]["loss"]).sum()
    return np.float32(total / 32.0)


# revision 11
# speedup vs baseline: 1.0005x; 1.0005x over previous
"""BiLSTM-CRF NLL loss on 8 Trainium2 NeuronCores.

Sharding: core c owns sequences [4c, 4c+4); each core runs BOTH LSTM
directions locally (fwd + bwd chains, concurrently scheduled) and the full
CRF for its 4 sequences. No collectives; host sums 8 per-core partials.

Recurrence: gates on PSUM partitions (gate-unit-major), batch on the free
dim, so each matmul moves only 4 columns. Per step/chain: 2 PSUM tiles
(g | i,f,o), each ONE accumulation group (start marks the whole 2KB zero
region): bias + W_ih-x mms (phase A, emitted first so they execute in the
previous step's tail) then W_hh-h mms (phase B), then Act sigmoid(i,f,o) +
tanh(g) -> DVE cell update -> Act tanh(c) -> DVE h. h lands in unit-major
layout (no transposes), stored fp8e4m3. Weights fp8e4m3; x/h matmuls use
fp8 DoubleRow (2 K-chunks per instruction). Step 0 skips h-mms (h0=0).

CRF partition function: exp-domain, two-sided. alpha ascends t=0..127 while
beta descends t=255..127 (128 links; beta stays in PSUM between steps);
they run concurrently and meet exactly at K=127: Z = sum_i a_K(i) b_K(i).
bf16 chain operands. Gold-path score via one-hot matmuls. Loss exits as
[1,4] per-core partials.

Self-contained: hardcodes all shapes; only needs numpy + concourse (+ml_dtypes).
"""
import numpy as np
import ml_dtypes

import concourse.bass as bass
import concourse.bacc as bacc
import concourse.tile as tile
from concourse import mybir
from concourse.tile_rust import add_dep_helper
from concourse.bass_utils import run_bass_kernel_spmd

F32 = mybir.dt.float32
FP8 = mybir.dt.float8e4
BF16 = mybir.dt.bfloat16
I32 = mybir.dt.int32
AF = mybir.ActivationFunctionType
ALU = mybir.AluOpType

B, S, E, H, T, V = 32, 256, 256, 512, 45, 50000
NS = 4                 # seqs per core
N = S * NS             # 1024 emission cols, n = 4t+s
NCH = 16               # gate chunks (2048/128)
HC = 4                 # h chunks (512/128)
SW = HC * NS           # state cols per step = 16
NB_T = 4               # transform n-blocks (of 64 steps = 256 cols each)
TBLK = S // NB_T       # 64 steps per transform block
LN45 = float(np.log(45.0))

_cached = {}


def _build(stop_after=None):
    lv = {"xf": 1, "rec": 2, "em": 3, "crf": 4, None: 5}[stop_after]
    nc = bacc.Bacc("TRN2", target_bir_lowering=False, debug=False, num_devices=8)

    d = {}
    d["emb"] = nc.dram_tensor("emb", [V, E], BF16, kind="ExternalInput")
    d["xidx"] = nc.dram_tensor("xidx", [128, 8], I32, kind="ExternalInput")
    d["wihf"] = nc.dram_tensor("wihf", [128, 32 * 128], FP8, kind="ExternalInput")
    d["wihb"] = nc.dram_tensor("wihb", [128, 32 * 128], FP8, kind="ExternalInput")
    d["whhf"] = nc.dram_tensor("whhf", [128, 64 * 128], FP8, kind="ExternalInput")
    d["whhb"] = nc.dram_tensor("whhb", [128, 64 * 128], FP8, kind="ExternalInput")
    d["biasf"] = nc.dram_tensor("biasf", [1, 2048], F32, kind="ExternalInput")
    d["biasb"] = nc.dram_tensor("biasb", [1, 2048], F32, kind="ExternalInput")
    d["linT"] = nc.dram_tensor("linT", [128, 8 * T], BF16, kind="ExternalInput")
    d["linb"] = nc.dram_tensor("linb", [T, 1], F32, kind="ExternalInput")
    d["id128"] = nc.dram_tensor("id128", [128, 128], F32, kind="ExternalInput")
    d["idbf"] = nc.dram_tensor("idbf", [128, 128], BF16, kind="ExternalInput")
    d["trans"] = nc.dram_tensor("trans", [T, T], F32, kind="ExternalInput")
    d["stend"] = nc.dram_tensor("stend", [T, 2], F32, kind="ExternalInput")
    d["oh"] = nc.dram_tensor("oh", [T, N], F32, kind="ExternalInput")
    d["oh2"] = nc.dram_tensor("oh2", [T, N], F32, kind="ExternalInput")
    d_loss = nc.dram_tensor("loss", [1, NS], F32, kind="ExternalOutput")

    with tile.TileContext(nc) as tc:
        with tc.tile_pool(name="persist", bufs=1) as pp, \
             tc.tile_pool(name="gxp", bufs=1) as gxp:
            # persistent weights / tables
            wih = {0: pp.tile([128, 32 * 128], FP8, tag="wihf", name="wihf"),
                   1: pp.tile([128, 32 * 128], FP8, tag="wihb", name="wihb")}
            whh = {0: pp.tile([128, 64 * 128], FP8, tag="whhf", name="whhf"),
                   1: pp.tile([128, 64 * 128], FP8, tag="whhb", name="whhb")}
            bias = {0: pp.tile([1, 2048], F32, tag="biasf", name="biasf"),
                    1: pp.tile([1, 2048], F32, tag="biasb", name="biasb")}
            ones1 = pp.tile([1, NS], F32, tag="ones1")
            id128 = pp.tile([128, 128], F32, tag="id128")
            idbf = pp.tile([128, 128], BF16, tag="idbf")
            xidx = pp.tile([128, 8], I32, tag="xidx")
            linT = pp.tile([128, 8 * T], BF16, tag="linT")
            nc.sync.dma_start(out=xidx[:], in_=d["xidx"][:])
            nc.sync.dma_start(out=id128[:], in_=d["id128"][:])
            nc.sync.dma_start(out=idbf[:], in_=d["idbf"][:])
            nc.vector.memset(ones1[:], 1.0)

            # XT block tiles: [nb] -> [128, 2 ec x 256 n] bf16
            xt = {nb: gxp.tile([128, 2 * TBLK * NS], FP8, tag=f"xt{nb}", name=f"xt{nb}")
                  for nb in range(NB_T)}
            # h state (unit-major): slot p in 0..255 = position, slot 256 = h0
            hsT = {0: pp.tile([128, SW * (S + 1)], FP8, tag="hsTf", name="hsTf"),
                   1: pp.tile([128, SW * (S + 1)], FP8, tag="hsTb", name="hsTb")}
            nc.vector.memset(hsT[0][:, SW * S: SW * (S + 1)], 0.0)
            nc.vector.memset(hsT[1][:, SW * S: SW * (S + 1)], 0.0)

            # ---------- phase 0: gather + transpose -> XT ----------
            with tc.tile_pool(name="gat", bufs=3) as gp, \
                 tc.tile_pool(name="ps_tp", bufs=4, space="PSUM") as ps_tp:
                for b in range(8):
                    X = gp.tile([128, E], BF16, tag="X")
                    nc.gpsimd.indirect_dma_start(
                        out=X[:],
                        out_offset=None,
                        in_=d["emb"][:],
                        in_offset=bass.IndirectOffsetOnAxis(ap=xidx[:, b:b + 1], axis=0),
                    )
                    nb, off = b // 2, (b % 2) * 128
                    for ec in range(2):
                        tp = ps_tp.tile([128, 128], BF16, tag="tp")
                        nc.tensor.transpose(tp[:], X[:, 128 * ec: 128 * ec + 128], idbf[:])
                        nc.vector.tensor_copy(
                            xt[nb][:, TBLK * NS * ec + off: TBLK * NS * ec + off + 128],
                            tp[:])

            # weight DMAs after the gathers so they share the DMA engines
            nc.sync.dma_start(out=wih[0][:], in_=d["wihf"][:])
            nc.sync.dma_start(out=wih[1][:], in_=d["wihb"][:])
            nc.sync.dma_start(out=whh[0][:], in_=d["whhf"][:])
            nc.sync.dma_start(out=whh[1][:], in_=d["whhb"][:])
            nc.sync.dma_start(out=bias[0][:], in_=d["biasf"][:])
            nc.sync.dma_start(out=bias[1][:], in_=d["biasb"][:])
            nc.sync.dma_start(out=linT[:], in_=d["linT"][:])

            # ---------- recurrence (x-transform fused into gate matmuls) ----------
            if lv == 1:
                probe = pp.tile([1, NS], F32, tag="probe")
                nc.vector.tensor_copy(probe[:], xt[0][0:1, 0:NS])
                nc.sync.dma_start(out=d_loss[:], in_=probe[:])
            if True:
                if lv >= 2:
                    with tc.tile_pool(name="rec0", bufs=6) as rp0, \
                         tc.tile_pool(name="rec1", bufs=6) as rp1, \
                         tc.tile_pool(name="psg0", bufs=2, space="PSUM") as pg0, \
                         tc.tile_pool(name="psg1", bufs=2, space="PSUM") as pg1, \
                         tc.tile_pool(name="psi0", bufs=2, space="PSUM") as pi0, \
                         tc.tile_pool(name="psi1", bufs=2, space="PSUM") as pi1:
                        rp = [rp0, rp1]
                        pg = [pg0, pg1]
                        pi = [pi0, pi1]
                        cprev = [None, None]
                        for dd in (0, 1):
                            cinit = rp[dd].tile([128, SW], BF16, tag="c")
                            nc.vector.memset(cinit[:], 0.0)
                            cprev[dd] = cinit

                        def rstep(dd, u):
                            # slot map:
                            # fwd: h_f(u) -> slot u; reads h_f(u-1) at slot u-1
                            #      (u=0 reads slot S = zeros)
                            # bwd: h_b(p=S-1-u) -> slot p; reads slot p+1
                            #      (u=0 reads slot S = zeros)
                            if dd == 0:
                                slot_w = u
                                slot_r = S if u == 0 else u - 1
                                col = u            # gx col index (timestep)
                            else:
                                p = S - 1 - u
                                slot_w = p
                                slot_r = S if u == 0 else p + 1
                                col = p
                            nb, j = col // TBLK, col % TBLK
                            xtb = xt[nb]
                            hprev = hsT[dd][:, SW * slot_r: SW * slot_r + SW]
                            Gg = pg[dd].tile([128, SW], F32, tag="Gg")
                            Gifo = pi[dd].tile([128, 3 * SW], F32, tag="Gi")

                            # gate chunks: i=0:4, f=4:8, o=8:12 (Gifo tile,
                            # cols 4*m), g=12:16 (Gg tile). One PSUM
                            # accumulation group per physical tile (= one 2KB
                            # zero region): start=True only on the tile's very
                            # first mm, stop=True on its very last; all other
                            # mms accumulate (first touch of each address
                            # replaces, since start marks the whole region
                            # pending-zero). Phase A (bias + W_ih x, no h dep)
                            # is emitted before phase B (W_hh h) so it can run
                            # in the previous step's tail; add_dep_helper pins
                            # start-first / stop-last against scheduler
                            # reordering. u=0: h_prev = 0, phase B skipped.
                            tiles = [(Gg, list(range(12, 16))),
                                     (Gifo, list(range(12)))]
                            DR = mybir.MatmulPerfMode.DoubleRow
                            xtv = xtb.rearrange("p (e c) -> p e c", e=2)
                            tile_mms = [[] for _ in tiles]
                            for gi, (dst, mlist) in enumerate(tiles):
                                for pos, m in enumerate(mlist):
                                    sl = dst[:, NS * pos: NS * pos + NS]
                                    mm = nc.tensor.matmul(
                                        sl, bias[dd][:, 128 * m: 128 * (m + 1)],
                                        ones1[:], start=(pos == 0), stop=False)
                                    tile_mms[gi].append(mm)
                                    # both E-chunks in one DoubleRow mm
                                    wpair = wih[dd][:, 2 * m * 128:
                                                    (2 * m + 2) * 128]
                                    mm = nc.tensor.matmul(
                                        sl,
                                        wpair.rearrange("p (c f) -> p c f", c=2),
                                        xtv[:, :, NS * j: NS * j + NS],
                                        start=False,
                                        stop=(u == 0 and pos == len(mlist) - 1),
                                        perf_mode=DR)
                                    tile_mms[gi].append(mm)
                            if u > 0:
                                for gi, (dst, mlist) in enumerate(tiles):
                                    for pos, m in enumerate(mlist):
                                        sl = dst[:, NS * pos: NS * pos + NS]
                                        for kp in range(HC // 2):
                                            # k-chunks 2kp, 2kp+1 in one mm
                                            hpair = whh[dd][
                                                :, (4 * m + 2 * kp) * 128:
                                                (4 * m + 2 * kp + 2) * 128]
                                            mm = nc.tensor.matmul(
                                                sl,
                                                hpair.rearrange(
                                                    "p (c f) -> p c f", c=2),
                                                hprev[:, 8 * kp: 8 * kp + 8]
                                                .rearrange("p (c s) -> p c s",
                                                           c=2),
                                                start=False,
                                                stop=(pos == len(mlist) - 1
                                                      and kp == HC // 2 - 1),
                                                perf_mode=DR)
                                            tile_mms[gi].append(mm)
                            for mms in tile_mms:
                                first, last = mms[0], mms[-1]
                                for mm in mms[1:]:
                                    add_dep_helper(mm.ins, first.ins, sync=False,
                                                   reason="group start first")
                                for mm in mms[:-1]:
                                    add_dep_helper(last.ins, mm.ins, sync=False,
                                                   reason="group stop last")
                            SGg = rp[dd].tile([128, SW], BF16, tag="SGg")
                            SGifo = rp[dd].tile([128, 3 * SW], BF16, tag="SGi")
                            nc.scalar.activation(SGg[:], Gg[:], AF.Tanh)
                            nc.scalar.activation(SGifo[:], Gifo[:], AF.Sigmoid)
                            t1 = rp[dd].tile([128, SW], BF16, tag="t1")
                            t2 = rp[dd].tile([128, SW], BF16, tag="t2")
                            th = rp[dd].tile([128, SW], BF16, tag="th")
                            cnext = rp[dd].tile([128, SW], BF16, tag="c")
                            nc.vector.tensor_mul(t1[:], SGifo[:, SW:2 * SW], cprev[dd][:])
                            nc.vector.tensor_mul(t2[:], SGifo[:, 0:SW], SGg[:])
                            nc.vector.tensor_add(cnext[:], t1[:], t2[:])
                            nc.scalar.activation(th[:], cnext[:], AF.Tanh)
                            nc.vector.tensor_mul(
                                hsT[dd][:, SW * slot_w: SW * slot_w + SW],
                                SGifo[:, 2 * SW:3 * SW], th[:])
                            cprev[dd] = cnext

                        for u in range(S):
                            rstep(0, u)
                            rstep(1, u)

                    if lv == 2:
                        probe = pp.tile([1, NS], F32, tag="probe")
                        nc.vector.tensor_copy(probe[:], hsT[0][0:1, 0:NS])
                        nc.sync.dma_start(out=d_loss[:], in_=probe[:])

            # ---------- emissions ----------
            em_lin = pp.tile([T, N], F32, tag="em_lin")
            exp_em = pp.tile([T, N], F32, tag="exp_em")
            if lv >= 3:
                with tc.tile_pool(name="emc", bufs=1) as ec_, \
                     tc.tile_pool(name="ps_em", bufs=2, space="PSUM") as ps_em:
                    linb = ec_.tile([T, 1], F32, tag="linb")
                    nc.sync.dma_start(out=linb[:], in_=d["linb"][:])
                    hv = {dd: hsT[dd].rearrange("p (t k s) -> p t k s", k=HC, s=NS)
                          for dd in (0, 1)}
                    for nb in range(2):
                        pe = ps_em.tile([T, 512], F32, tag="pe")
                        toff = nb * 128
                        for kc in range(8):
                            dd, k = kc // 4, kc % 4
                            # rhs: hsT[dd] cols {16*(toff+t) + 4k + s}, 512 free
                            rhs = hv[dd][:, toff:toff + 128, k:k + 1, :]
                            nc.tensor.matmul(
                                pe[:], linT[:, T * kc: T * (kc + 1)], rhs,
                                start=(kc == 0), stop=(kc == 7))
                        nc.vector.tensor_scalar_add(
                            em_lin[:, 512 * nb: 512 * (nb + 1)], pe[:], linb[:])
                        nc.scalar.activation(exp_em[:, 512 * nb: 512 * (nb + 1)],
                                             pe[:], AF.Exp, bias=linb[:])
                if lv == 3:
                    probe = pp.tile([1, NS], F32, tag="probe")
                    nc.vector.tensor_copy(probe[:], em_lin[0:1, 0:NS])
                    nc.sync.dma_start(out=d_loss[:], in_=probe[:])

            # ---------- CRF ----------
            if lv >= 4:
                with tc.tile_pool(name="crf", bufs=1) as cp, \
                     tc.tile_pool(name="qs", bufs=3) as qp, \
                     tc.tile_pool(name="ps_q", bufs=2, space="PSUM") as ps_q:
                    trans_sb = cp.tile([T, T], F32, tag="trans")
                    stend = cp.tile([T, 2], F32, tag="stend")
                    Ep = cp.tile([T, T], F32, tag="Ep")
                    estart = cp.tile([T, 1], F32, tag="estart")
                    eend = cp.tile([T, 1], F32, tag="eend")
                    nln45 = cp.tile([T, 1], F32, tag="nln45")
                    ones45 = cp.tile([T, 1], F32, tag="ones45")
                    oh = cp.tile([T, N], F32, tag="oh")
                    oh2 = cp.tile([T, N], F32, tag="oh2")
                    nc.sync.dma_start(out=trans_sb[:], in_=d["trans"][:])
                    nc.sync.dma_start(out=stend[:], in_=d["stend"][:])
                    nc.sync.dma_start(out=oh[:], in_=d["oh"][:])
                    nc.sync.dma_start(out=oh2[:], in_=d["oh2"][:])
                    nc.vector.memset(nln45[:], -LN45)
                    nc.vector.memset(ones45[:], 1.0)
                    nc.scalar.activation(Ep[:], trans_sb[:], AF.Exp, bias=nln45[:])
                    nc.scalar.activation(estart[:], stend[:, 0:1], AF.Exp)
                    nc.scalar.activation(eend[:], stend[:, 1:2], AF.Exp)

                    # partition function via two-sided vector chains that
                    # meet at K=127:  Z = sum_i alpha_K(i) * beta_K(i).
                    # alpha ascends from t=0, beta descends from t=255
                    # (beta_255 = exp(end_t)); both run concurrently, halving
                    # the serial chain length. bf16 operands (4x cheaper PE
                    # streaming; ~1e-4 relative effect on logZ).
                    K = 127
                    Epb = cp.tile([T, T], BF16, tag="Epb")
                    nc.scalar.activation(Epb[:], trans_sb[:], AF.Exp, bias=nln45[:])
                    EpbT = cp.tile([T, T], BF16, tag="EpbT")
                    with tc.tile_pool(name="ps_t", bufs=1, space="PSUM") as ps_t:
                        tpt = ps_t.tile([T, T], BF16, tag="tpt")
                        nc.tensor.transpose(tpt[:], Epb[:], idbf[0:T, 0:T])
                        nc.vector.tensor_copy(EpbT[:], tpt[:])

                    q = qp.tile([T, NS], BF16, tag="q")
                    nc.vector.tensor_scalar_mul(q[:], exp_em[:, 0:NS], estart[:])
                    bq0 = qp.tile([T, NS], BF16, tag="bq")
                    nc.vector.tensor_scalar_mul(
                        bq0[:], eend[:].to_broadcast([T, NS]), ones45[:])
                    bq = bq0                     # beta lives in PSUM after j=1
                    with tc.tile_pool(name="ps_b", bufs=2, space="PSUM") as ps_b:
                        for j in range(1, K + 1):
                            # alpha: t = j
                            sA = ps_q.tile([T, NS], F32, tag="sA")
                            nc.tensor.matmul(sA[:], Epb[:], q[:],
                                             start=True, stop=True)
                            qn = qp.tile([T, NS], BF16, tag="q")
                            nc.vector.tensor_mul(
                                qn[:], sA[:], exp_em[:, NS * j: NS * (j + 1)])
                            q = qn
                            # beta: t = 255 - j; wv = e_{t+1} (.) beta_{t+1};
                            # beta_t = Ep @ wv  (lhsT = Ep^T); beta stays in
                            # PSUM, read directly by the next step's DVE mul
                            t_ = S - 1 - j
                            wv = qp.tile([T, NS], BF16, tag="wv")
                            nc.vector.tensor_mul(
                                wv[:], bq[:],
                                exp_em[:, NS * (t_ + 1): NS * (t_ + 2)])
                            sB = ps_b.tile([T, NS], F32, tag="sB")
                            nc.tensor.matmul(sB[:], EpbT[:], wv[:],
                                             start=True, stop=True)
                            bq = sB
                        # S-1=255 links is odd: one extra beta step so beta
                        # reaches position K (= alpha's position) exactly
                        wv = qp.tile([T, NS], BF16, tag="wv")
                        nc.vector.tensor_mul(
                            wv[:], bq[:], exp_em[:, NS * (K + 1): NS * (K + 2)])
                        sB = ps_b.tile([T, NS], F32, tag="sB")
                        nc.tensor.matmul(sB[:], EpbT[:], wv[:],
                                         start=True, stop=True)
                        bqf = cp.tile([T, NS], F32, tag="bqf")
                        nc.vector.tensor_copy(bqf[:], sB[:])
                        bq = bqf
                    if lv == 4:
                        probe = pp.tile([1, NS], F32, tag="probe")
                        nc.vector.tensor_copy(probe[:], q[0:1, :])
                        nc.sync.dma_start(out=d_loss[:], in_=probe[:])

                    if lv >= 5:
                        w = cp.tile([T, NS], F32, tag="w")
                        logZ = cp.tile([1, NS], F32, tag="logZ")
                        em_h = cp.tile([1, 2 * NS], F32, tag="em_h")
                        tr_h = cp.tile([1, 2 * NS], F32, tag="tr_h")
                        em_sc = cp.tile([1, NS], F32, tag="em_sc")
                        tr_sc = cp.tile([1, NS], F32, tag="tr_sc")
                        sten_s = cp.tile([1, NS], F32, tag="sten_s")
                        nc.vector.tensor_mul(w[:], q[:], bq[:])
                        with tc.tile_pool(name="ps_f", bufs=1, space="PSUM") as ps_f:
                            sumw = ps_f.tile([1, NS], F32, tag="f1")
                            nc.tensor.matmul(sumw[:], ones45[:], w[:],
                                             start=True, stop=True)
                            nc.scalar.activation(logZ[:], sumw[:], AF.Ln)

                            S1 = cp.tile([T, N], F32, tag="S1")
                            nc.vector.tensor_mul(S1[:], em_lin[:], oh[:])
                            S2 = cp.tile([T, N], F32, tag="S2")
                            for ck in range(2):
                                sl = slice(512 * ck, 512 * (ck + 1))
                                s1p = ps_f.tile([1, 512], F32, tag="fbig")
                                nc.tensor.matmul(s1p[:], ones45[:], S1[:, sl],
                                                 start=True, stop=True)
                                nc.vector.tensor_reduce(
                                    em_h[:, NS * ck: NS * (ck + 1)],
                                    s1p.rearrange("p (t b) -> p b t", b=NS),
                                    axis=mybir.AxisListType.X, op=ALU.add)
                                Rp_ = ps_f.tile([T, 512], F32, tag="fR")
                                nc.tensor.matmul(Rp_[:], trans_sb[:], oh[:, sl],
                                                 start=True, stop=True)
                                nc.vector.tensor_mul(S2[:, sl], Rp_[:], oh2[:, sl])
                                s2p = ps_f.tile([1, 512], F32, tag="fbig2")
                                nc.tensor.matmul(s2p[:], ones45[:], S2[:, sl],
                                                 start=True, stop=True)
                                nc.vector.tensor_reduce(
                                    tr_h[:, NS * ck: NS * (ck + 1)],
                                    s2p.rearrange("p (t b) -> p b t", b=NS),
                                    axis=mybir.AxisListType.X, op=ALU.add)
                            nc.vector.tensor_add(em_sc[:], em_h[:, 0:NS],
                                                 em_h[:, NS:2 * NS])
                            nc.vector.tensor_add(tr_sc[:], tr_h[:, 0:NS],
                                                 tr_h[:, NS:2 * NS])

                            stp = cp.tile([T, NS], F32, tag="stp")
                            enp = cp.tile([T, NS], F32, tag="enp")
                            nc.vector.tensor_scalar_mul(stp[:], oh[:, 0:NS],
                                                        stend[:, 0:1])
                            nc.vector.tensor_scalar_mul(enp[:], oh[:, N - NS:N],
                                                        stend[:, 1:2])
                            sten = ps_f.tile([1, NS], F32, tag="f2")
                            nc.tensor.matmul(sten[:], ones45[:], stp[:],
                                             start=True, stop=False)
                            nc.tensor.matmul(sten[:], ones45[:], enp[:],
                                             start=False, stop=True)
                            nc.vector.tensor_copy(sten_s[:], sten[:])

                        sc1 = cp.tile([1, NS], F32, tag="sc1")
                        sc2 = cp.tile([1, NS], F32, tag="sc2")
                        lossa = cp.tile([1, NS], F32, tag="lossa")
                        lossb = cp.tile([1, NS], F32, tag="lossb")
                        nc.vector.tensor_add(sc1[:], em_sc[:], tr_sc[:])
                        nc.vector.tensor_add(sc2[:], sc1[:], sten_s[:])
                        nc.vector.tensor_tensor(out=lossa[:], in0=logZ[:],
                                                in1=sc2[:], op=ALU.subtract)
                        nc.scalar.activation(lossb[:], lossa[:], AF.Copy,
                                             bias=(S - 1) * LN45)
                        nc.sync.dma_start(out=d_loss[:], in_=lossb[:])

    nc.finalize()
    return nc


def _pack_wT(w, kchunks):
    # w: [M_out=2048-ish rows (gate units, reordered), K] ->
    # [128, (nm*kchunks)*128] tiles: tile (m*kchunks+ec) = w[mU, ecK].T
    M, K = w.shape
    nm = M // 128
    assert K == 128 * kchunks
    tiles = []
    for m in range(nm):
        for ec in range(kchunks):
            blk = w[m * 128:(m + 1) * 128, ec * 128:(ec + 1) * 128]
            tiles.append(np.ascontiguousarray(blk.T))
    return np.concatenate(tiles, axis=1)


def _perm_gates_ifog(w):
    # torch gate order i,f,g,o (blocks of H) -> our chunk order i,f,o,g
    i, f, g, o = np.split(w, 4, axis=0)
    return np.concatenate([i, f, o, g], axis=0)


def prepare_in_maps(**inputs):
    x = np.asarray(inputs["x"]).astype(np.int32)          # [32, 256]
    tags = np.asarray(inputs["tags"]).astype(np.int32)
    emb = np.asarray(inputs["emb"], dtype=np.float32)
    lin_w = np.asarray(inputs["lin_w"], dtype=np.float32)
    lin_b = np.asarray(inputs["lin_b"], dtype=np.float32)
    start_t = np.asarray(inputs["start_t"], dtype=np.float32)
    end_t = np.asarray(inputs["end_t"], dtype=np.float32)
    trans = np.asarray(inputs["trans"], dtype=np.float32)

    wihp = {0: _perm_gates_ifog(np.asarray(inputs["w_ih_f"], np.float32)),
            1: _perm_gates_ifog(np.asarray(inputs["w_ih_b"], np.float32))}
    whhp = {0: _perm_gates_ifog(np.asarray(inputs["w_hh_f"], np.float32)),
            1: _perm_gates_ifog(np.asarray(inputs["w_hh_b"], np.float32))}
    bp = {0: _perm_gates_ifog(np.asarray(inputs["b_f"], np.float32)),
          1: _perm_gates_ifog(np.asarray(inputs["b_b"], np.float32))}

    wih_t = {dd: _pack_wT(wihp[dd], 2).astype(ml_dtypes.float8_e4m3) for dd in (0, 1)}
    whh_t = {dd: _pack_wT(whhp[dd], 4).astype(ml_dtypes.float8_e4m3) for dd in (0, 1)}
    bias_t = {dd: bp[dd].reshape(1, 2048) for dd in (0, 1)}

    # linT [128, 8*T]: tile kc = lin_w[:, kc*128:(kc+1)*128].T (fwd 0-3, bwd 4-7)
    lin_tiles = [np.ascontiguousarray(lin_w[:, kc * 128:(kc + 1) * 128].T)
                 for kc in range(8)]
    linT = np.concatenate(lin_tiles, axis=1).astype(ml_dtypes.bfloat16)

    id128 = np.eye(128, dtype=np.float32)

    in_maps = []
    for core in range(8):
        seqs = slice(4 * core, 4 * core + 4)
        xs = x[seqs]                                      # [4, 256]
        # xidx [128, 16]: col b, row r -> x[s=(r%4), t=(128b+r)//4]
        nflat = xs.T.reshape(-1)                          # n = 4t+s
        xidx = np.ascontiguousarray(nflat.reshape(8, 128).T).astype(np.int32)

        tg = tags[seqs]                                   # [4, 256]
        oh = np.zeros((T, N), np.float32)
        oh[tg.T.reshape(-1), np.arange(N)] = 1.0
        oh2 = np.zeros((T, N), np.float32)
        oh2[:, 0:N - NS] = oh[:, NS:N]

        in_maps.append({
            "emb": emb.astype(ml_dtypes.bfloat16),
            "xidx": xidx,
            "wihf": wih_t[0], "wihb": wih_t[1],
            "whhf": whh_t[0], "whhb": whh_t[1],
            "biasf": bias_t[0], "biasb": bias_t[1],
            "linT": linT,
            "linb": lin_b.reshape(T, 1),
            "id128": id128,
            "idbf": np.eye(128, dtype=ml_dtypes.bfloat16),
            "trans": trans,
            "stend": np.stack([start_t, end_t], axis=1),
            "oh": oh,
            "oh2": oh2,
        })
    return in_maps


def get_nc():
    if "nc" not in _cached:
        _cached["nc"] = _build()
    return _cached["nc"]


def kernel(**inputs):
    in_maps = prepare_in_maps(**inputs)
    res = run_bass_kernel_spmd(get_nc(), in_maps, core_ids=list(range(8)))
    total = np.float64(0.0)
    for core in range(8):
        total += np.float64(res.results[core]["loss"]).sum()
    return np.float32(total / 32.0)
